# revision 41
# baseline (speedup 1.0000x reference)
"""Trainium2 Bass kernel for nn_LongDistanceAttention (GNN message passing).

v3 strategy (8 NeuronCores, SPMD, node/row sharding):
  - Host marshalling only: A cast to fp8 and replicated to every core in the
    mask-stream tile layout (no device AllGather of A); X pre-transposed;
    M0u (= A^T local block, fp16 0/1) and M0p (= i-pair packed A^T, fp8
    {0,1,8,9}) pre-tiled on host.
  - mask1 (A^2 reach) DR matmuls interleaved with the stage-1 GAT loops so
    the PE works through the collective start barrier; mask2 right after;
    only collective is the small gw (g + WaT) AllGather.
  - Unpack of packed mask PSUM via single-op mod/is_ge chains; M1p pack-add
    on gpsimd; Wh PSUM evacuation on the scalar (ACT) engine.
  - stage-1 leaky-relu+exp on ACT (Lrelu then Exp); masks stored fp16
    (fast DVE multiplies); M2u fp8 (SBUF budget); expS bf16 (range).
  - hops 1-3 fused in one pass over j-chunks; matmuls grouped by PSUM bank;
    row-sum denominators batched; reciprocal_approx_fast for softmax denoms.
"""

import sys

import ml_dtypes
import numpy as np

sys.path.insert(0, "/opt/trn_rl_repo")

import concourse.bass as bass  # noqa: E402
import concourse.mybir as mybir  # noqa: E402
import concourse.tile as tile  # noqa: E402
from concourse import bacc  # noqa: E402
from concourse.bass_utils import run_bass_kernel_spmd  # noqa: E402
from concourse.masks import make_identity  # noqa: E402

P = 128
N = 4096
NB = N // P            # 32 j-chunks
HID = 256
OUT_DIM = 128
NCORES = 8
LOC = N // NCORES      # 512 local rows per core
LB = LOC // P          # 4 local partition chunks
HLOC = LOC // 2        # 256 packed i-pairs
ALPHA = 0.2
MG2 = 8                # 512-wide column strips of A
KO = 4                 # k-subchunks per core (512/128)

F32 = mybir.dt.float32
I16 = mybir.dt.int16
BF16 = mybir.dt.bfloat16
F16 = mybir.dt.float16
FP8 = mybir.dt.float8e4
DR = mybir.MatmulPerfMode.DoubleRow
AF = mybir.ActivationFunctionType
OP = mybir.AluOpType

_CACHE = {}
last_in_maps = None


def build_kernel():
    nc = bacc.Bacc(
        "TRN2",
        target_bir_lowering=False,
        debug=False,
        enable_asserts=False,
        num_devices=NCORES,
    )

    # ---- kernel I/O ----
    XT_d = nc.dram_tensor("XT", [HID, N], F16, kind="ExternalInput")
    XlT_d = nc.dram_tensor("XlocT", [HID, LOC], F16, kind="ExternalInput")
    A8_d = nc.dram_tensor("A8f", [NCORES * MG2 * P, KO * 512], FP8,
                          kind="ExternalInput")
    M0u_d = nc.dram_tensor("M0u", [P, NB * LOC], F16, kind="ExternalInput")
    M0p_d = nc.dram_tensor("M0p", [P, NB * HLOC], FP8, kind="ExternalInput")
    Ws_d = nc.dram_tensor("Ws16", [HID, HID], F16, kind="ExternalInput")
    WsT_d = nc.dram_tensor("WsT16", [HID, HID], F16, kind="ExternalInput")
    r_d = nc.dram_tensor("r", [2 * HID, 1], F32, kind="ExternalInput")
    Wl_d = nc.dram_tensor("Wl16", [HID, HID], F16, kind="ExternalInput")
    Wo_d = nc.dram_tensor("Wo16", [HID, OUT_DIM], F16, kind="ExternalInput")
    bo_d = nc.dram_tensor("b_out", [OUT_DIM], F32, kind="ExternalInput")
    out_d = nc.dram_tensor("out", [OUT_DIM, LOC], F32, kind="ExternalOutput")
    DBG = bool(int(__import__("os").environ.get("DBG_DUMP", "0")))
    if DBG:
        dbg_m0 = nc.dram_tensor("dbg_m0", [N, LOC], F32, kind="ExternalOutput")
        dbg_m1 = nc.dram_tensor("dbg_m1", [N, LOC], F32, kind="ExternalOutput")
        dbg_m2 = nc.dram_tensor("dbg_m2", [N, LOC], F32, kind="ExternalOutput")
        dbg_es = nc.dram_tensor("dbg_es", [N, LOC], F32, kind="ExternalOutput")
        dbg_ht = nc.dram_tensor("dbg_ht", [HID, LOC], F32, kind="ExternalOutput")

    # ---- internal DRAM for the h-exchange collective ----
    GW = LOC * OUT_DIM + HID * LOC  # f16 elements: g part + watT part
    gw_loc = nc.dram_tensor("gw_loc", [GW], F16)
    gw_all = nc.dram_tensor("gw_all", [NCORES * GW], F16, addr_space="Shared")
    groups = [list(range(NCORES))]

    a8v = A8_d.ap().rearrange(
        "(c mg p) (ko j) -> c mg p ko j", mg=MG2, p=P, j=512
    )

    with tile.TileContext(nc) as tc:
        with (
            tc.tile_pool(name="const", bufs=1) as cpool,
            tc.tile_pool(name="small", bufs=1) as sm,
            tc.tile_pool(name="maskp", bufs=1) as mp,
            tc.tile_pool(name="wk", bufs=1) as wk,
            tc.tile_pool(name="pp", bufs=1, space="PSUM") as pp,
        ):
            # =========== constants / weights ===========
            ident = cpool.tile([P, P], F32)
            make_identity(nc, ident)
            ident_h = cpool.tile([P, P], F16)
            nc.vector.tensor_copy(ident_h[:], ident[:])

            ones1_h = cpool.tile([1, P], F16)
            nc.vector.memset(ones1_h[:], 1.0)
            ones1_f = cpool.tile([1, P], F32)
            nc.vector.memset(ones1_f[:], 1.0)
            onesF = cpool.tile([P, P], F32)
            nc.vector.memset(onesF[:], 1.0)
            sevens_i = cpool.tile([P, 2, HLOC], I16)
            nc.vector.memset(sevens_i[:], 7)
            zeros_i = cpool.tile([P, 2, HLOC], I16)
            nc.vector.memset(zeros_i[:], 0)
            sevens_v = sevens_i
            zeros_v = zeros_i
            onz_h = cpool.tile([P, 1], F16)
            nc.vector.memset(onz_h[:], 1.0)
            onz_b = cpool.tile([P, 1], BF16)
            nc.vector.memset(onz_b[:], 1.0)

            r_f = cpool.tile([P, 4], F32)
            nc.scalar.dma_start(
                r_f[:], r_d.ap().rearrange("(c p) o -> p (c o)", p=P)
            )
            r_h = cpool.tile([P, 4], F16)
            nc.vector.tensor_copy(r_h[:], r_f[:])
            rph = r_h.rearrange("p (h c) -> p c h", c=2)
            bo_sb = cpool.tile([P, 1], F32)
            nc.scalar.dma_start(bo_sb[:], bo_d.ap().rearrange("(o p) -> p o", p=P))
            # fp16 weights DMA'd straight from host-cast inputs
            Ws_aug = cpool.tile([P, 2, HID + 2], F16)
            WsT_h = cpool.tile([P, 2, HID], F16)
            Wl_h = cpool.tile([P, 2, HID], F16)
            Wo_h = cpool.tile([P, 2, OUT_DIM], F16)
            for W_d, Wdst in (
                (WsT_d, WsT_h[:]),
                (Ws_d, Ws_aug[:, :, 0:HID]),
                (Wl_d, Wl_h[:]),
                (Wo_d, Wo_h[:]),
            ):
                nc.scalar.dma_start(
                    Wdst, W_d.ap().rearrange("(kh p) n -> p kh n", p=P)
                )
            # w12 = W_s @ [r1 r2]  (cols 256/257 of the augmented W_s)
            w12 = cpool.tile([P, 2, 2], F32)
            w12h = cpool.tile([P, 2, 2], F16)
            for mc in range(2):
                pw12 = pp.tile([P, LOC], F32, tag="big", bufs=1, name="pw12")
                for kc in range(2):
                    nc.tensor.matmul(
                        pw12[:, 0:2],
                        WsT_h[:, kc, mc * P: (mc + 1) * P],
                        rph[:, kc, :],
                        start=(kc == 0),
                        stop=(kc == 1),
                    )
                nc.vector.tensor_copy(w12[:, mc], pw12[:, 0:2])
                nc.vector.tensor_copy(w12h[:, mc], pw12[:, 0:2])
            nc.vector.tensor_copy(Ws_aug[:, :, HID: HID + 2], w12[:])

            # ---- persistent tiles ----
            M0u = mp.tile([P, NB, LOC], F16, name="M0u")
            M1u = mp.tile([P, NB, LOC], FP8, name="M1u")
            M1p = mp.tile([P, NB, HLOC], FP8, name="M1p")
            M2u = mp.tile([P, NB, LOC], FP8, name="M2u")
            # pre-touch wk ring tags so the stack allocator places them
            # below the 'early' pool (lazy first-use would trap its space)
            wk.tile([P, KO, 512], FP8, tag="a8s", bufs=(9 if DBG else 11), name="a8s")
            wk.tile([P, 2, HLOC], F16, tag="t8", bufs=1, name="t8")
            wk.tile([P, 2, HLOC], I16, tag="vi", bufs=2, name="vi")
            wk.tile([P, 2, HLOC], I16, tag="va", bufs=2, name="va")
            wk.tile([P, LOC], F16, tag="s1", bufs=7, name="s1")
            wk.tile([P, LOC], F32, tag="nrm", bufs=2, name="nrm")
            wk.tile([P, 3, LOC], BF16, tag="ek", bufs=5, name="ek")
            hT = sm.tile([P, 2, LOC], F16, name="hT")
            s_nat = sm.tile([P, NB], F32, name="s_nat")
            outT = sm.tile([P, LOC], F32, name="outT")
            B_sb = sm.tile([P, LOC], F16, name="B_sb")

            M1v = M1u.rearrange("p jc (q par) -> p jc par q", par=2)
            M2v = M2u.rearrange("p jc (q par) -> p jc par q", par=2)

            # early pool: XT + Wh, freed before mask2's extra buffers
            earlycm = tc.tile_pool(name="early", bufs=1)
            early = earlycm.__enter__()
            XT_sb = early.tile([P, 2, N], F16, name="XT_sb")
            Wh_aug = early.tile([P, NB, HID + 2], F16, name="Wh_aug")
            XlT_sb = early.tile([P, 2, LOC], F16, name="XlT_sb")
            M0p = early.tile([P, NB, HLOC], FP8, name="M0p")

            # input loads (scalar queue; a8 stream rides the sync queue)
            nc.scalar.dma_start(
                XT_sb[:], XT_d.ap().rearrange("(kh p) n -> p kh n", p=P)
            )
            nc.scalar.dma_start(
                XlT_sb[:], XlT_d.ap().rearrange("(kh p) n -> p kh n", p=P)
            )
            nc.scalar.dma_start(
                M0p[:], M0p_d.ap().rearrange("p (jc q) -> p jc q", q=HLOC)
            )
            m0src = M0u_d.ap().rearrange("p (jc i) -> p jc i", i=LOC)
            for q4 in range(4):
                nc.scalar.dma_start(
                    M0u[:, q4 * 8: (q4 + 1) * 8, :], m0src[:, q4 * 8: (q4 + 1) * 8, :]
                )

            # =========== mask matmul group helper ===========
            def mask_group(g, rhs_p, out_v, out_p, scope):
                """One 512-wide column strip of A^T @ rhs_p (i-pair packed,
                fp8 DoubleRow). Produces jc chunks 4g..4g+3 of out."""
                with nc.named_scope(scope):
                    tiles = []
                    for c in range(NCORES):
                        t = wk.tile([P, KO, 512], FP8,
                                    tag="a8s", bufs=(9 if DBG else 11),
                                    name="a8s")
                        nc.sync.dma_start(t[:], a8v[c, g])
                        tiles.append(t)
                    for half in range(2):
                        pms = pp.tile([P, LOC], F32, tag="pm", bufs=2,
                                      name="pms")
                        for mi in (2 * half, 2 * half + 1):
                            col = (mi % 2) * HLOC
                            for c in range(NCORES):
                                for t2 in range(2):
                                    kc = KO * c + 2 * t2
                                    nc.tensor.matmul(
                                        pms[:, col: col + HLOC],
                                        tiles[c][:, 2 * t2: 2 * t2 + 2,
                                                 mi * P: (mi + 1) * P],
                                        rhs_p[:, kc: kc + 2, :],
                                        start=(c == 0 and t2 == 0),
                                        stop=(c == NCORES - 1 and t2 == 1),
                                        perf_mode=DR,
                                    )
                        jc0 = 4 * g + 2 * half
                        pv = pms.rearrange("p (jc q) -> p jc q", q=HLOC)
                        # even lane: (v & 7) > 0 ; odd lane: v >= 7.5
                        vi = wk.tile([P, 2, HLOC], I16, tag="vi", bufs=2,
                                     name="vi")
                        nc.vector.tensor_copy(vi[:], pv[:])
                        va = wk.tile([P, 2, HLOC], I16, tag="va", bufs=2,
                                     name="va")
                        nc.vector.tensor_scalar(
                            va[:], vi[:], 7, None, OP.bitwise_and
                        )
                        nc.vector.tensor_scalar(
                            out_v[:, jc0: jc0 + 2, 0, :], va[:], 0, None,
                            OP.is_gt,
                        )
                        nc.vector.tensor_scalar(
                            out_v[:, jc0: jc0 + 2, 1, :], pv[:], 7.5, None,
                            OP.is_ge,
                        )
                        if out_p is not None:
                            t8 = wk.tile([P, 2, HLOC], F16, tag="t8", bufs=1,
                                         name="t8")
                            nc.vector.tensor_scalar(
                                t8[:], pv[:], 7.5, 8.0, OP.is_ge, OP.mult,
                            )
                            nc.gpsimd.tensor_add(
                                out=out_p[:, jc0: jc0 + 2, :], in0=t8[:],
                                in1=out_v[:, jc0: jc0 + 2, 0, :],
                            )

            # =========== phase B: s_i row -> B_sb ===========
            with nc.named_scope("prepS"):
                psi = pp.tile([P, LOC], F32, tag="accz", bufs=1, name="psi")
                for kh in range(2):
                    nc.tensor.matmul(
                        psi[0:1, :],
                        w12h[:, kh, 0:1],
                        XlT_sb[:, kh, :],
                        start=(kh == 0),
                        stop=(kh == 1),
                    )
                si_h = sm.tile([1, LOC], F16, name="si_h")
                nc.vector.tensor_copy(si_h[:], psi[0:1, :])
                psB = pp.tile([P, LOC], F32, tag="big", bufs=1, name="psB")
                nc.tensor.matmul(psB[:], ones1_h[:], si_h[:], start=True,
                                 stop=True)
                nc.vector.tensor_copy(B_sb[:], psB[:])

            # =========== phase C0: mask1 head start ===========
            mask_group(0, M0p, M1v, M1p, "mask1")
            mask_group(1, M0p, M1v, M1p, "mask1")

            # =========== phase C1: Wh = X @ [W_s | W_s r] + mask1 b ======
            with nc.named_scope("stage1w"):
                for o in range(NB):
                    pa = pp.tile([P, LOC], F32, tag="pa", bufs=1, name="pa")
                    for kh in range(2):
                        nc.tensor.matmul(
                            pa[:, 0: HID + 2],
                            XT_sb[:, kh, o * P: (o + 1) * P],
                            Ws_aug[:, kh, :],
                            start=(kh == 0),
                            stop=(kh == 1),
                        )
                    nc.scalar.activation(
                        Wh_aug[:, o, :], pa[:, 0: HID + 2], AF.Copy
                    )
                    nc.scalar.activation(
                        s_nat[:, o: o + 1], pa[:, HID + 1: HID + 2], AF.Copy
                    )
                    if o % 8 == 3:
                        mask_group(2 + o // 8, M0p, M1v, M1p, "mask1")
                mask_group(6, M0p, M1v, M1p, "mask1")
                mask_group(7, M0p, M1v, M1p, "mask1")

            # =========== phase C2: stage-1 attention blended w/ mask2 ====
            with nc.named_scope("stage1"):
                u0 = pp.tile([P, LOC], F32, tag="acc0", bufs=1, name="u0")
                u1 = pp.tile([P, LOC], F32, tag="acc1", bufs=1, name="u1")
                uza = pp.tile([P, LOC], F32, tag="accz", bufs=1, name="uza")
                for o in range(NB):
                    t1 = wk.tile([P, LOC], F16, tag="s1", bufs=7, name="t1")
                    nc.vector.tensor_scalar(
                        t1[:], B_sb[:], s_nat[:, o: o + 1], None, OP.add,
                    )
                    # exp(leaky_relu(x)) == max(exp(x), exp(alpha*x))
                    ee0 = wk.tile([P, LOC], F16, tag="s1", bufs=7, name="ee0")
                    nc.scalar.activation(ee0[:], t1[:], AF.Exp)
                    ee1 = wk.tile([P, LOC], F16, tag="s1", bufs=7, name="ee1")
                    nc.scalar.activation(ee1[:], t1[:], AF.Exp, scale=ALPHA)
                    ee = wk.tile([P, LOC], F16, tag="s1", bufs=7, name="ee")
                    nc.vector.tensor_max(out=ee[:], in0=ee0[:], in1=ee1[:])
                    em = wk.tile([P, LOC], F16, tag="s1", bufs=7, name="em")
                    nc.vector.tensor_mul(out=em[:], in0=ee[:], in1=M0u[:, o])
                    last = o == NB - 1
                    nc.tensor.matmul(u0[:], Wh_aug[:, o, 0:P], em[:],
                                     start=(o == 0), stop=last)
                    nc.tensor.matmul(u1[:], Wh_aug[:, o, P: 2 * P], em[:],
                                     start=(o == 0), stop=last)
                    nc.tensor.matmul(uza[0:1, :], onz_h[:], em[:],
                                     start=(o == 0), stop=last)
                    if o % 4 == 3 and o < 24:
                        mask_group(o // 4, M1p, M2v, None, "mask2")
                # normalize + gelu -> h^T (fp16)
                zrs = sm.tile([1, LOC], F32, name="zrs")
                nc.vector.reciprocal_approx_fast(out=zrs[:], in_=uza[0:1, :])
                psZ = pp.tile([P, LOC], F32, tag="big", bufs=1, name="psZ")
                nc.tensor.matmul(psZ[:], ones1_f[:], zrs[:], start=True,
                                 stop=True)
                zb = wk.tile([P, LOC], F32, tag="nrm", bufs=2, name="zb")
                nc.vector.tensor_copy(zb[:], psZ[:])
                for mt, um in enumerate((u0, u1)):
                    tn = wk.tile([P, LOC], F32, tag="nrm", bufs=2, name="tn")
                    nc.vector.tensor_mul(out=tn[:], in0=um[:], in1=zb[:])
                    nc.scalar.activation(hT[:, mt], tn[:], AF.Gelu)

            # =========== phase D: WaT / G + AllGather ===========
            with nc.named_scope("gathers"):
                watT_sb = sm.tile([P, 2, LOC], F16, name="watT_sb")
                for m2 in range(2):
                    psW = pp.tile([P, LOC], F32, tag="pm", bufs=2, name="psW")
                    for f in range(2):
                        nc.tensor.matmul(
                            psW[:],
                            Wl_h[:, f, m2 * P: (m2 + 1) * P],
                            hT[:, f, :],
                            start=(f == 0),
                            stop=(f == 1),
                        )
                    nc.vector.tensor_copy(watT_sb[:, m2], psW[:])
                nc.scalar.dma_start(
                    gw_loc.ap()[LOC * OUT_DIM: GW]
                    .rearrange("(p kh i) -> p kh i", p=P, i=LOC),
                    watT_sb[:],
                )
                # G^T = W_out^T @ h^T ; outT init = G^T
                psG = pp.tile([P, LOC], F32, tag="pm", bufs=2, name="psG")
                for f in range(2):
                    nc.tensor.matmul(
                        psG[:],
                        Wo_h[:, f, :],
                        hT[:, f, :],
                        start=(f == 0),
                        stop=(f == 1),
                    )
                nc.vector.tensor_copy(outT[:], psG[:])
                GT_h = sm.tile([P, LOC], F16, name="GT_h")
                nc.vector.tensor_copy(GT_h[:], psG[:])
                g_nat = sm.tile([P, LB, OUT_DIM], BF16, name="g_nat")
                tph = pp.tile([P, 2, P], F16, tag="tph", bufs=1, name="tph")
                for ic in range(LB):
                    nc.tensor.transpose(
                        tph[:, ic % 2, :],
                        GT_h[:, ic * P: (ic + 1) * P],
                        ident_h[:],
                    )
                    nc.vector.tensor_copy(g_nat[:, ic], tph[:, ic % 2, :])
                nc.scalar.dma_start(
                    gw_loc.ap()[0: LOC * OUT_DIM]
                    .rearrange("(p ic f) -> p ic f", p=P, f=OUT_DIM)
                    .bitcast(BF16),
                    g_nat[:],
                )
                nc.gpsimd.collective_compute(
                    "AllGather", OP.bypass,
                    ins=[gw_loc[:]], outs=[gw_all[:]],
                    replica_groups=groups,
                )

            # mask2 tail groups fill the PE while the AllGather runs
            mask_group(6, M1p, M2v, None, "mask2")
            mask_group(7, M1p, M2v, None, "mask2")

            # free XT/Wh; allocate late tiles
            earlycm.__exit__(None, None, None)
            mp2cm = tc.tile_pool(name="mp2", bufs=1)
            mp2 = mp2cm.__enter__()
            expS = mp2.tile([P, NB, LOC], BF16, name="expS")
            Gall = mp2.tile([P, NB, OUT_DIM], BF16, name="Gall")
            WaTall = mp2.tile([P, 2 * NCORES, LOC], F16, name="WaTall")
            with nc.named_scope("gathers2"):
                gwv = gw_all.ap().rearrange("(c e) -> c e", e=GW)
                for cc in range(NCORES):
                    nc.scalar.dma_start(
                        Gall[:, cc * LB: (cc + 1) * LB, :],
                        gwv[cc, 0: LOC * OUT_DIM]
                        .rearrange("(p ic f) -> p ic f", p=P, f=OUT_DIM)
                        .bitcast(BF16),
                    )
                    nc.scalar.dma_start(
                        WaTall[:, 2 * cc: 2 * cc + 2, :],
                        gwv[cc, LOC * OUT_DIM: GW]
                        .rearrange("(p kh i) -> p kh i", p=P, i=LOC),
                    )

            # =========== phase G: scores ===========
            with nc.named_scope("scores"):
                for m in range(NB):
                    c, mi = divmod(m, LB)
                    pst = pp.tile([P, LOC], F32, tag="pm", bufs=2,
                                  name="pst")
                    for f in range(2):
                        nc.tensor.matmul(
                            pst[:],
                            WaTall[:, 2 * c + f, mi * P: (mi + 1) * P],
                            hT[:, f, :],
                            start=(f == 0),
                            stop=(f == 1),
                        )
                    nc.scalar.activation(expS[:, m], pst[:], AF.Exp)

            # =========== phase H: fused hops 1-3 ===========
            with nc.named_scope("hops"):
                uG1 = pp.tile([P, LOC], F32, tag="acc0", bufs=1, name="uG1")
                uG2 = pp.tile([P, LOC], F32, tag="acc1", bufs=1, name="uG2")
                uG3 = pp.tile([P, LOC], F32, tag="pa", bufs=1, name="uG3")
                # one bank for all three row-sum series, col-tiled at
                # partitions 0/32/64 so the three matmuls run concurrently
                uzz = pp.tile([P, LOC], F32, tag="accz", bufs=1, name="uzz")
                for mb in range(NB // 4):
                    eks = []
                    for m4 in range(4):
                        m = 4 * mb + m4
                        ekb = wk.tile([P, 3, LOC], BF16, tag="ek", bufs=5,
                                      name="ekb")
                        nc.vector.tensor_mul(
                            out=ekb[:, 0], in0=expS[:, m], in1=M0u[:, m]
                        )
                        nc.gpsimd.tensor_mul(
                            out=ekb[:, 1], in0=expS[:, m], in1=M1u[:, m]
                        )
                        nc.vector.tensor_mul(
                            out=ekb[:, 2], in0=expS[:, m], in1=M2u[:, m]
                        )
                        eks.append((m, ekb))
                    first = mb == 0
                    last = mb == NB // 4 - 1
                    for k, acc in ((0, uG1), (1, uG2), (2, uG3)):
                        for i4, (m, ekb) in enumerate(eks):
                            nc.tensor.matmul(
                                acc[:], Gall[:, m, :], ekb[:, k],
                                start=(first and i4 == 0),
                                stop=(last and i4 == 3),
                            )
                    for i4, (m, ekb) in enumerate(eks):
                        for k in range(3):
                            nc.tensor.matmul(
                                uzz[32 * k: 32 * k + 1, :], onz_b[:],
                                ekb[:, k],
                                start=(first and i4 == 0),
                                stop=(last and i4 == 3),
                                tile_position=(0, 32 * k),
                            )
                # normalization: copy each D row to SBUF (partition-matched),
                # broadcast via a ones row at the same base partition,
                # reciprocal at base 0, then scale-accumulate
                zrawt = sm.tile([P, LOC], F32, name="zrawt")
                for k in range(3):
                    nc.vector.tensor_copy(
                        zrawt[32 * k: 32 * k + 1, :],
                        uzz[32 * k: 32 * k + 1, :],
                    )
                for k, acc in ((0, uG1), (1, uG2), (2, uG3)):
                    psZh = pp.tile([P, LOC], F32, tag="big", bufs=1,
                                   name="psZh")
                    nc.tensor.matmul(
                        psZh[:], onesF[32 * k: 32 * k + 1, :],
                        zrawt[32 * k: 32 * k + 1, :],
                        start=True, stop=True,
                    )
                    zbh = wk.tile([P, LOC], F32, tag="nrm", bufs=2,
                                  name="zbh")
                    nc.vector.reciprocal_approx_fast(out=zbh[:], in_=psZh[:])
                    tn = wk.tile([P, LOC], F32, tag="nrm", bufs=2, name="tnh")
                    nc.vector.tensor_mul(out=tn[:], in0=acc[:], in1=zbh[:])
                    nc.vector.tensor_add(out=outT[:], in0=outT[:], in1=tn[:])

            if DBG:
                for dbg_d, tile_src, nchunks in (
                    (dbg_m0, M0u, NB), (dbg_m1, M1u, NB), (dbg_m2, M2u, NB),
                    (dbg_es, expS, NB), (dbg_ht, hT, 2),
                ):
                    dv = dbg_d.ap().rearrange("(o p) i -> p o i", p=P)
                    for o in range(nchunks):
                        dt_ = wk.tile([P, LOC], F32, tag="dbg", bufs=1,
                                      name="dbg")
                        nc.vector.tensor_copy(dt_[:], tile_src[:, o])
                        nc.scalar.dma_start(dv[:, o], dt_[:])

            # =========== output: bias + store ===========
            with nc.named_scope("out"):
                yt = sm.tile([P, LOC], F32, name="yt")
                nc.vector.tensor_scalar(
                    yt[:], outT[:], bo_sb[:, 0:1], None, OP.add
                )
                nc.scalar.dma_start(out_d[:, :], yt[:])

            mp2cm.__exit__(None, None, None)

    nc.compile()
    return nc


def _get_nc():
    if "nc" not in _CACHE:
        _CACHE["nc"] = build_kernel()
    return _CACHE["nc"]


def kernel(X, A, W_s, r, W_l, W_out, b_out):
    global last_in_maps
    Xf = np.asarray(X, dtype=np.float32)
    XT16 = np.ascontiguousarray(Xf.T, dtype=np.float16)
    Af = np.asarray(A, dtype=np.float32)
    A8 = Af.astype(ml_dtypes.float8_e4m3fn)
    # full A, replicated, in the mask-stream tile layout:
    # a8f[(c*MG2+mg)*P+p, ko*512+j] = A[c*512+ko*128+p, mg*512+j]
    a8f = np.ascontiguousarray(
        A8.reshape(NCORES, KO, P, MG2, 512).transpose(0, 3, 2, 1, 4)
        .reshape(NCORES * MG2 * P, KO * 512)
    )
    in_maps = []
    for c in range(NCORES):
        blk = Af[c * LOC: (c + 1) * LOC]          # [512, 4096]
        m0u = np.ascontiguousarray(
            blk.T.reshape(NB, P, LOC).transpose(1, 0, 2).reshape(P, NB * LOC),
            dtype=np.float16,
        )
        p0 = (blk[0::2] + 8.0 * blk[1::2]).T       # [4096, 256]
        m0p = np.ascontiguousarray(
            p0.reshape(NB, P, HLOC).transpose(1, 0, 2).reshape(P, NB * HLOC)
        ).astype(ml_dtypes.float8_e4m3fn)
        in_maps.append(
            {
                "XT": XT16,
                "XlocT": np.ascontiguousarray(XT16[:, c * LOC: (c + 1) * LOC]),
                "A8f": a8f,
                "M0u": m0u,
                "M0p": m0p,
                "Ws16": np.ascontiguousarray(W_s, dtype=np.float16),
                "WsT16": np.ascontiguousarray(
                    np.asarray(W_s, dtype=np.float32).T, dtype=np.float16),
                "r": np.ascontiguousarray(r, dtype=np.float32),
                "Wl16": np.ascontiguousarray(W_l, dtype=np.float16),
                "Wo16": np.ascontiguousarray(W_out, dtype=np.float16),
                "b_out": np.ascontiguousarray(b_out, dtype=np.float32),
            }
        )
    last_in_maps = in_maps
    nc = _get_nc()
    res = run_bass_kernel_spmd(nc, in_maps, core_ids=list(range(NCORES)))
    Y = np.empty((N, OUT_DIM), dtype=np.float32)
    for c in range(NCORES):
        Y[c * LOC: (c + 1) * LOC, :] = res.results[c]["out"].T
    return Y


if __name__ == "__main__":
    build_kernel()
    print("build OK")
